# revision 24
# baseline (speedup 1.0000x reference)
"""Trainium2 Bass kernel for CustomAttention (B=4, N=2048, C=1024, H=16).

Sharding: 8-way tensor-parallel over heads (2 heads per core, all batches).
Each core computes qkv for its 2 heads, full attention for its (batch, head)
pairs, and a partial output projection over its 128 channels. Host sums the
8 partial projections and adds proj_b.

Structure (round 3):
  - fp16 everywhere off-chip; softmax bias folded multiplicatively via
    host-shipped expb = exp(bias^T - 6) fp16; device: e = exp(s - 6) * expb.
  - Attention loops batch-outer: per (nch, h, b), 8 key-block PAIRS; each
    pair shares one 2-bank PSUM tile, one wide exp, one wide multiply with a
    contiguous expb slab slice, and two PV matmuls (ones-column augmented so
    PSUM row 64 accumulates the softmax denominator).
  - PV matmuls and the normalization chain are software-pipelined one pair
    behind the score matmuls (pend crosses (h,b) boundaries), so the PE never
    heads-of-line-blocks on the exp/mul chain and norms spread out instead of
    bunching at head boundaries.
  - Normalization: DVE reciprocal straight from PSUM row 64, SP-queue (HWDGE)
    DMA to partition 0, gpsimd partition_broadcast, one DVE multiply.
  - Projection groups (2 output chunks each) drain one per pair-iteration as
    soon as both heads of a (nch, b) are normalized.
  - Phase-1 elementwise work (qk bias add, paired V copies) runs on ACT.
"""

import sys

if "/opt/trn_rl_repo" not in sys.path:
    sys.path.insert(0, "/opt/trn_rl_repo")

import numpy as np

B, N, C, H, D = 4, 2048, 1024, 16, 64
T = B * N  # 8192
HPC = 2  # heads per core
NCORES = 8
MB = N // 128  # 16 key blocks per batch
NCH = N // 512  # 4 query chunks per batch
TC_ = T // 512  # 16 token chunks (qkv phase)
KC = C // 128  # 8 contraction chunks (qkv phase)
JC = C // 128  # 8 output-channel chunks (proj phase)
SHIFT = 6.0  # softmax stability shift, folded into host-side exp(bias - SHIFT)
VA = D + 1  # 65: per-head v block width incl. denominator ones column

_CACHE = {}


def build_nc():
    import concourse.bacc as bacc
    import concourse.mybir as mybir
    import concourse.tile as tile
    from contextlib import ExitStack

    F32 = mybir.dt.float32
    F16 = mybir.dt.float16
    EXP = mybir.ActivationFunctionType.Exp

    nc = bacc.Bacc(None, target_bir_lowering=False)
    xT = nc.dram_tensor("xT", [C, T], F16, kind="ExternalInput")
    wqk = nc.dram_tensor("wqk", [C, 2 * HPC * D], F16, kind="ExternalInput")
    wv = nc.dram_tensor("wv", [C, 2 * VA], F16, kind="ExternalInput")
    bqk = nc.dram_tensor("bqk", [2 * HPC * D], F32, kind="ExternalInput")
    bv = nc.dram_tensor("bv", [2 * VA], F16, kind="ExternalInput")
    expb = nc.dram_tensor("expb", [HPC, N, N], F16, kind="ExternalInput")
    pw = nc.dram_tensor("pw", [HPC * D, C], F16, kind="ExternalInput")
    outT = nc.dram_tensor("outT", [C, T], F16, kind="ExternalOutput")

    with tile.TileContext(nc) as tc, ExitStack() as ctx:
        sing = ctx.enter_context(tc.tile_pool(name="sing", bufs=1))
        ps = ctx.enter_context(tc.tile_pool(name="ps", bufs=1, space="PSUM"))
        wk = ctx.enter_context(tc.tile_pool(name="wk", bufs=1))

        # ---- residents ----
        bqk_sb = sing.tile([128, 2], F32)
        nc.sync.dma_start(out=bqk_sb, in_=bqk.rearrange("(m p) -> p m", m=2))
        bv_sb = sing.tile([1, 2 * VA], F16)
        nc.sync.dma_start(out=bv_sb, in_=bv.rearrange("(p m) -> p m", p=1))
        ones_sb = sing.tile([1, 128], F16)
        nc.vector.memset(ones_sb, 1.0)
        mshift = sing.tile([128, 1], F32)
        nc.vector.memset(mshift, -SHIFT)
        wqk_sb = sing.tile([128, KC, 2 * HPC * D], F16)
        nc.sync.dma_start(out=wqk_sb, in_=wqk.rearrange("(k p) m -> p k m", p=128))
        wv_sb = sing.tile([128, KC, 2 * VA], F16)
        nc.sync.dma_start(out=wv_sb, in_=wv.rearrange("(k p) m -> p k m", p=128))
        pw_sb = sing.tile([128, C], F16)
        nc.sync.dma_start(out=pw_sb, in_=pw[:, :])

        qT = sing.tile([128, T], F16)  # rows: q_h0 d0..63 | q_h1 d0..63
        kT = sing.tile([128, T], F16)
        v_aug = sing.tile([128, B, MB, 2 * VA], F16)
        attn_oT = sing.tile([128, B, N], F16)

        def sm_tile(name):
            return ps.tile([128, 1024], F32, tag="sm", bufs=3, name=name)

        eslabs = {}

        def get_eslab(nch, h):
            if (nch, h) not in eslabs:
                t = wk.tile(
                    [128, MB * 512], F16, tag="eslab", bufs=2, name=f"es_{nch}_{h}"
                )
                for mb in range(MB):
                    nc.sync.dma_start(
                        out=t[:, mb * 512 : (mb + 1) * 512],
                        in_=expb[h, mb * 128 : (mb + 1) * 128, nch * 512 : nch * 512 + 512],
                    )
                eslabs[(nch, h)] = t
            return eslabs[(nch, h)]

        # ---- phase 1 emitters (interleaved with attention below) ----
        def emit_phase1_chunk(t):
            t0 = t * 512
            x_tiles = []
            for kc in range(KC):
                x_t = wk.tile([128, 512], F16, tag="x", bufs=26, name=f"x_{t}_{kc}")
                nc.sync.dma_start(
                    out=x_t, in_=xT[kc * 128 : (kc + 1) * 128, t0 : t0 + 512]
                )
                x_tiles.append(x_t)
            smqk = sm_tile(f"qk_{t}")
            for m in range(2):  # q, k
                for kc in range(KC):
                    nc.tensor.matmul(
                        smqk[:, m * 512 : (m + 1) * 512],
                        wqk_sb[:, kc, m * 128 : (m + 1) * 128],
                        x_tiles[kc],
                        start=(kc == 0),
                        stop=(kc == KC - 1),
                    )
            nc.scalar.add(qT[:, t0 : t0 + 512], smqk[:, 0:512], bqk_sb[:, 0:1])
            nc.scalar.add(kT[:, t0 : t0 + 512], smqk[:, 512:1024], bqk_sb[:, 1:2])
            b_idx, off = divmod(t, TC_ // B)
            for jp in range(2):  # pairs of 128-token blocks
                smv = sm_tile(f"v_{t}_{jp}")
                for jj in range(2):
                    j = 2 * jp + jj
                    c0 = jj * 512
                    for kc in range(KC):
                        nc.tensor.matmul(
                            smv[:, c0 : c0 + 2 * VA],
                            x_tiles[kc][:, j * 128 : (j + 1) * 128],
                            wv_sb[:, kc, :],
                            start=(kc == 0),
                            stop=False,
                        )
                    nc.tensor.matmul(
                        smv[:, c0 : c0 + 2 * VA], ones_sb, bv_sb, start=False, stop=True
                    )
                mb0 = off * 4 + 2 * jp
                nc.scalar.copy(
                    v_aug[:, b_idx, mb0 : mb0 + 2, :],
                    smv[:, :].rearrange("p (j x) -> p j x", j=2)[:, :, 0 : 2 * VA],
                )

        # ---- attention/projection emitters ----
        pend = []  # (opv, nch, h, b, mb, e_w), PV lags scores by PV_LAG pairs
        PV_LAG = 3
        proj_q = []  # ready projection groups: (nch, b, jcp, alt)
        norm_ctr = [0]

        def emit_norm(opv_t, nch, h, b):
            n0 = nch * 512
            r0 = wk.tile([VA, 512], F32, tag="r0", bufs=2, name=f"r0_{nch}_{h}_{b}")
            nc.vector.reciprocal(r0[D : D + 1, :], opv_t[D : D + 1, :])
            d0 = wk.tile([1, 512], F32, tag="d0", bufs=2, name=f"d0_{nch}_{h}_{b}")
            nc.gpsimd.dma_start(out=d0, in_=r0[D : D + 1, :])
            rbc = wk.tile([D, 512], F32, tag="rbc", bufs=2, name=f"rb_{nch}_{h}_{b}")
            nc.gpsimd.partition_broadcast(rbc, d0)
            if h == 0:
                nc.vector.tensor_mul(
                    attn_oT[0:D, b, n0 : n0 + 512], opv_t[0:D, :], rbc
                )
            else:
                ot = wk.tile([D, 512], F16, tag="ot", bufs=2, name=f"ot_{nch}_{b}")
                nc.vector.tensor_mul(ot, opv_t[0:D, :], rbc)
                nc.gpsimd.dma_start(out=attn_oT[D : 2 * D, b, n0 : n0 + 512], in_=ot)
                for jcp in range(4):
                    proj_q.append((nch, b, jcp, (norm_ctr[0] + jcp) % 2 == 0))
                norm_ctr[0] += 1

        def flush_pend():
            if not pend:
                return
            opv_t, nchp, hp, bp, mb, e_w = pend.pop(0)
            for q in range(4):
                nc.tensor.matmul(
                    opv_t,
                    v_aug[:, bp, mb + q, hp * VA : (hp + 1) * VA],
                    e_w[:, q * 512 : (q + 1) * 512],
                    start=(mb + q == 0),
                    stop=(mb + q == MB - 1),
                )
            if mb + 3 == MB - 1:
                emit_norm(opv_t, nchp, hp, bp)

        def emit_proj(nch, b, jcp, alt):
            n0 = nch * 512
            bo = b * N
            smt = sm_tile(f"pj_{nch}_{b}_{jcp}")
            for i in range(2):
                jc = 2 * jcp + i
                nc.tensor.matmul(
                    smt[:, i * 512 : (i + 1) * 512],
                    pw_sb[:, jc * 128 : (jc + 1) * 128],
                    attn_oT[:, b, n0 : n0 + 512],
                    start=True,
                    stop=True,
                )
            o_sb = wk.tile([128, 1024], F16, tag="o", bufs=3, name=f"ob_{nch}_{b}_{jcp}")
            nc.vector.tensor_copy(o_sb, smt)
            for i in range(2):
                jc = 2 * jcp + i
                nc.scalar.dma_start(
                    out=outT[jc * 128 : (jc + 1) * 128, bo + n0 : bo + n0 + 512],
                    in_=o_sb[:, i * 512 : (i + 1) * 512],
                )

        opv_cur = [None]

        def emit_group(nch, h, b, gp):
            # one group = 2 key-block pairs = 4 key blocks: two 2-bank score
            # tiles, two wide exps into one [128,2048] eraw, ONE wide multiply
            n0 = nch * 512
            hd = h * D
            bo = b * N
            eslab = get_eslab(nch, h)
            if gp == 0 and b == 0:
                if h == 0:
                    get_eslab(nch, 1)
                elif nch + 1 < NCH:
                    get_eslab(nch + 1, 0)
            if gp == 0:
                opv_cur[0] = ps.tile(
                    [VA, 512], F32, tag="opv", bufs=2, name=f"o_{nch}_{h}_{b}"
                )
            opv_t = opv_cur[0]
            mb = 4 * gp
            eraw = wk.tile(
                [128, 2048], F16, tag="eraw", bufs=2, name=f"er_{nch}_{h}_{b}_{gp}"
            )
            for half in range(2):
                smt = sm_tile(f"s_{nch}_{h}_{b}_{gp}_{half}")
                for i in range(2):
                    m0 = (mb + 2 * half + i) * 128
                    nc.tensor.matmul(
                        smt[:, i * 512 : (i + 1) * 512],
                        kT[hd : hd + D, bo + m0 : bo + m0 + 128],
                        qT[hd : hd + D, bo + n0 : bo + n0 + 512],
                        start=True,
                        stop=True,
                    )
                nc.scalar.activation(
                    eraw[:, half * 1024 : (half + 1) * 1024], smt, EXP,
                    bias=mshift[:, 0:1],
                )
            e_w = wk.tile(
                [128, 2048], F16, tag="e", bufs=4, name=f"e_{nch}_{h}_{b}_{gp}"
            )
            nc.vector.tensor_mul(e_w, eraw, eslab[:, mb * 512 : (mb + 4) * 512])
            pend.append((opv_t, nch, h, b, mb, e_w))
            if len(pend) > PV_LAG:
                flush_pend()
            # drain proj with a backlog so its matmuls never reach the PE
            # head before the producing norm chain has completed
            if len(proj_q) > 4:
                emit_proj(*proj_q.pop(0))
                if len(proj_q) > 8:
                    emit_proj(*proj_q.pop(0))

        # ---- fused emission schedule ----
        # Attention pairs for (nch, h, b) may only be emitted once phase-1
        # chunks for batch b (t = 4b..4b+3) are emitted; order heads/batches
        # so eslab prefetch and qkv production stay ahead of consumption.
        def attention_order():
            for nch in range(NCH):
                for h in range(HPC):
                    for b in range(B):
                        for gp in range(MB // 4):
                            yield (nch, h, b, gp)

        att = attention_order()
        emitted_done = False

        def emit_groups(k):
            nonlocal emitted_done
            for _ in range(k):
                nxt = next(att, None)
                if nxt is None:
                    emitted_done = True
                    return
                emit_group(*nxt)

        for t in range(TC_):
            emit_phase1_chunk(t)
            if t == 0:
                get_eslab(0, 0)  # first slab behind the first x chunk
            if t == 6:
                get_eslab(0, 1)  # queue slab (0,1) behind batch-1 x chunks
            if t >= 4:
                # interleave attention groups behind ready qkv chunks
                emit_groups(1 if t < 12 else 2)
        while not emitted_done:
            emit_groups(1)
        while pend:
            flush_pend()
        while proj_q:
            emit_proj(*proj_q.pop(0))

    nc.compile()
    return nc


def _get_nc():
    if "nc" not in _CACHE:
        _CACHE["nc"] = build_nc()
    return _CACHE["nc"]


def make_in_maps(x, attn_bias, qkv_w, qkv_b, proj_w):
    x = np.asarray(x, dtype=np.float32)
    attn_bias = np.asarray(attn_bias, dtype=np.float32)
    qkv_w = np.asarray(qkv_w, dtype=np.float32)
    qkv_b = np.asarray(qkv_b, dtype=np.float32)
    proj_w = np.asarray(proj_w, dtype=np.float32)

    xT = np.ascontiguousarray(x.reshape(T, C).T.astype(np.float16))
    # expb[h, m, n] = exp(bias[h, n, m] - SHIFT)
    expb_full = np.exp(
        attn_bias[0].transpose(0, 2, 1).astype(np.float64) - SHIFT
    ).astype(np.float16)
    scale = 1.0 / np.sqrt(D)

    in_maps = []
    for cid in range(NCORES):
        h0 = HPC * cid
        rows = slice(h0 * D, (h0 + HPC) * D)  # 128 contiguous head dims
        wq = qkv_w[rows, :] * scale
        wk_ = qkv_w[C + h0 * D : C + (h0 + HPC) * D, :]
        wqk_c = np.ascontiguousarray(np.concatenate([wq, wk_], 0).T.astype(np.float16))
        wv_rows = qkv_w[2 * C + h0 * D : 2 * C + (h0 + HPC) * D, :]  # [128, 1024]
        wv_c = np.zeros((C, 2 * VA), np.float16)
        wv_c[:, 0:D] = wv_rows[0:D, :].T
        wv_c[:, VA : VA + D] = wv_rows[D : 2 * D, :].T
        bq = qkv_b[rows] * scale
        bk = qkv_b[C + h0 * D : C + (h0 + HPC) * D]
        bqk_c = np.ascontiguousarray(np.concatenate([bq, bk], 0).astype(np.float32))
        bv_c = np.zeros((2 * VA,), np.float16)
        bv_c[0:D] = qkv_b[2 * C + h0 * D : 2 * C + h0 * D + D]
        bv_c[D] = 1.0
        bv_c[VA : VA + D] = qkv_b[2 * C + h0 * D + D : 2 * C + (h0 + 2) * D]
        bv_c[VA + D] = 1.0
        expb_c = np.ascontiguousarray(expb_full[h0 : h0 + HPC])
        pw_c = np.ascontiguousarray(
            proj_w[:, cid * 128 : (cid + 1) * 128].T.astype(np.float16)
        )
        in_maps.append(
            {
                "xT": xT,
                "wqk": wqk_c,
                "wv": wv_c,
                "bqk": bqk_c,
                "bv": bv_c,
                "expb": expb_c,
                "pw": pw_c,
            }
        )
    return in_maps


def combine_outputs(partials, proj_b):
    proj_b = np.asarray(proj_b, dtype=np.float32)
    acc = partials[0].astype(np.float32)
    for p in partials[1:]:
        acc += p.astype(np.float32)
    out = acc.T + proj_b[None, :]
    return np.ascontiguousarray(out.reshape(B, N, C).astype(np.float32))


def kernel(x, attn_bias, qkv_w, qkv_b, proj_w, proj_b):
    from concourse.bass_utils import run_bass_kernel_spmd

    in_maps = make_in_maps(x, attn_bias, qkv_w, qkv_b, proj_w)
    res = run_bass_kernel_spmd(_get_nc(), in_maps, core_ids=list(range(NCORES)))
    partials = [res.results[i]["outT"] for i in range(NCORES)]
    return combine_outputs(partials, proj_b)
